# revision 10
# baseline (speedup 1.0000x reference)
"""Single-head causal attention on 8 TRN2 NeuronCores.

Problem: x[B=8, T=2048, C=1024], Wq/Wk/Wv[C, H=64] (fp32)
  q = x@Wq; k = x@Wk; v = x@Wv
  wei = softmax(mask(q k^T * C^-0.5)); out = wei @ v       -> [B, T, H]

Sharding: data-parallel over batch, one batch element per core.  The
host hands each core its x slice already C-major (x[b].T) and the
weights packed per 128-c-tile, so the device reads x with the
contraction dim on partitions directly -- no on-device transpose of x.

Per-core dataflow (all matmuls bf16, fp32 PSUM accumulation):
  1. xT [C,T] fp32 --SWDGE cast DMA--> xt bf16 [128, 8, 2048],
     streamed in eight half-chunk pieces so QKV matmuls chase the load.
  2. QKV per 512-wide t-chunk: packed [Wq|Wk] stationary -> psum
     [qT;kT] (kT shifted to partitions 0:64 via the sync HWDGE queue,
     serialized with the xbar transposes), Wv -> vT -> xbar -> v_nat.
     Scale 1/sqrt(C) folds into the exp() activation.
  3. S^T tiles = kT.T @ qT (keys on partitions); causal mask added on
     diagonal tiles; exp on ScalarE (no max subtraction needed: logits
     are O(1) by construction); PV: out_un^T[65, T] accumulates
     [v|ones].T @ exp(S^T) -- row 64 = sumexp for free.
  4. PE-transpose out_un^T chunks, multiply by 1/sumexp, DMA out.
"""
import sys

sys.path.insert(0, "/opt/trn_rl_repo")

import numpy as np

import concourse.bass as bass
import concourse.mybir as mybir
import concourse.tile as tile
from concourse import bacc
from concourse.bass_utils import run_bass_kernel_spmd
from concourse.masks import make_identity

B, T, C, H = 8, 2048, 1024, 64
NTT = T // 128   # 16 t-tiles
NCT = C // 128   # 8  c-tiles
NCH = T // 512   # 4  t-chunks (moving free dim)
SCALE = float(C) ** -0.5
MASKVAL = -32768.0  # pre-scale additive mask; * SCALE -> -1024 -> exp -> 0
VP = 80          # v_nat per-tile stride: 160B, 32B-aligned for xbar transpose

F32 = mybir.dt.float32
BF16 = mybir.dt.bfloat16


def emit_loads(nc, xD, xtpool):
    # stream x in: SWDGE cast DMAs, half-chunk pieces so the QKV matmuls
    # chase the load front.  Issued before any other gpsimd work except
    # the warmup memset + weight load.
    xt = xtpool.tile([128, NCT, T], BF16, tag="xt")
    xR = xD.rearrange("(k p) t -> p k t", p=128)
    for n in range(NCH):
        sl = slice(n * 512, (n + 1) * 512)
        nc.gpsimd.dma_start(xt[:, 0:4, sl], xR[:, 0:4, sl])
        nc.gpsimd.dma_start(xt[:, 4:8, sl], xR[:, 4:8, sl])
    return xt


def emit_body(nc, tc, outD, consts, pools, xt):
    AF = mybir.ActivationFunctionType
    ALU = mybir.AluOpType
    wqkv, maskd, ident, dum = consts
    qkpool, ptpool, opool, fpool = pools

    # ---- QKV projections + attention, pipelined per 512-wide t-chunk ----
    qk_a = qkpool.tile([128, T], BF16, tag="qka")   # rows 0:64 qT, 64:128 kT
    kt_lo = qkpool.tile([64, T], BF16, tag="ktlo")  # kT at partitions 0:64
    vt = qkpool.tile([64, T], BF16, tag="vt")       # vT at partitions 0:64
    v_nat = qkpool.tile([128, NTT, VP], BF16, tag="vnat")  # [s_lo, s_hi, v|1]
    nc.gpsimd.memset(v_nat[:, :, H:H + 1], 1.0)
    o_out = fpool.tile([128, NTT, H], F32, tag="oout")
    outR = outD.rearrange("(g p) h -> p g h", p=128)
    with (
        tc.tile_pool(name="qkps", bufs=2, space="PSUM") as qkps,
        tc.tile_pool(name="aux", bufs=1, space="PSUM") as aux,
        tc.tile_pool(name="ops", bufs=2, space="PSUM") as ops,
        tc.tile_pool(name="stps", bufs=3, space="PSUM") as stps,
    ):
        vps = aux  # v-projection psum bank
        fps = stps  # fin transposes rotate through the S psum bufs
        # PE warm-up on the zero dummy: ramps the HAM clock-gate to 8/8
        # while the first x chunk is still in flight.
        warm = qkps.tile([128, 512], F32, tag="psqk")
        for _ in range(12):
            nc.tensor.matmul(
                warm[:], dum[:, 0:128], dum[:], start=True, stop=True
            )

        def emit_qkv(n):
            sl = slice(n * 512, (n + 1) * 512)
            # qk first: the attention-critical chain is qk_a -> kt_lo -> S
            ps_qk = qkps.tile([128, 512], F32, tag="psqk")
            for k in range(NCT):
                nc.tensor.matmul(
                    ps_qk[:], wqkv[:, k, 0:128], xt[:, k, sl],
                    start=(k == 0), stop=(k == NCT - 1),
                )
            nc.vector.tensor_copy(qk_a[:, sl], ps_qk[:])
            # kT shifted to partitions 0:64 (stationary base must match the
            # moving q base).  On the sync HWDGE queue it serializes with
            # the xbar transposes (same queue -> no corruption hazard) and
            # stays clear of the x-load FIFO on SWDGE.
            nc.sync.dma_start(kt_lo[:, sl], qk_a[64:128, sl])
            ps_v_t = vps.tile([128, 512], F32, tag="aux")
            ps_v = ps_v_t[0:64, :]
            for k in range(NCT):
                nc.tensor.matmul(
                    ps_v[:], wqkv[:, k, 128:192], xt[:, k, sl],
                    start=(k == 0), stop=(k == NCT - 1),
                )
            nc.vector.tensor_copy(vt[:, sl], ps_v[:])
            # per-chunk xbar transpose: vT[64h, 512s] -> v_nat[s_lo, tk, h]
            nc.sync.dma_start(
                v_nat[:, n * 4:(n + 1) * 4, 0:H], vt[:, sl], transpose=True
            )

        out_pcs = {}

        def emit_attn_core(ci):
            out_pc = ops.tile([H + 1, 512], F32, tag="outc")
            out_pcs[ci] = out_pc
            nsb = 4 * ci + 4
            pending = None  # software pipeline: PV(sb-1) emits after ST(sb)
            for sb in range(nsb):
                r = sb - 4 * ci  # >=0 on diagonal s-blocks
                t0 = max(r, 0) * 128
                tw = 512 - t0
                st = stps.tile([128, 512], F32, tag="st")
                nc.tensor.matmul(
                    st[:, :tw],
                    kt_lo[:, sb * 128:(sb + 1) * 128],
                    qk_a[0:64, ci * 512 + t0:(ci + 1) * 512],
                    start=True, stop=True,
                )
                if r >= 0:  # diagonal block: causal mask
                    nc.vector.tensor_tensor(
                        st[:, 0:128], st[:, 0:128], maskd[:], op=ALU.add
                    )
                pt = ptpool.tile([128, 512], BF16, tag="pt")
                nc.scalar.activation(pt[:, :tw], st[:, :tw], AF.Exp, scale=SCALE)
                if pending is not None:
                    nc.tensor.matmul(*pending[0], **pending[1])
                pending = (
                    (out_pc[:, t0:512], v_nat[:, sb, 0:H + 1], pt[:, :tw]),
                    dict(start=(sb == 0), stop=(sb == nsb - 1)),
                )
            nc.tensor.matmul(*pending[0], **pending[1])

        def emit_attn_out(ci):
            # normalize + transpose + store this chunk
            out_pc = out_pcs[ci]
            o_c = opool.tile([H + 1, 512], F32, tag="osb")
            nc.vector.tensor_copy(o_c[:], out_pc[:])
            last = ci == NCH - 1
            for rr in range(4):
                tk = ci * 4 + rr
                fin_t = fps.tile([128, 512], F32, tag="st")
                fin = fin_t[:, 0:H + 1]
                nc.tensor.transpose(
                    fin[:],
                    o_c[:, rr * 128:(rr + 1) * 128],
                    ident[0:H + 1, 0:H + 1],
                )
                rcp = fpool.tile([128, 1], F32, tag="rcp")
                nc.vector.reciprocal(rcp[:], fin[:, H:H + 1])
                nc.vector.tensor_scalar_mul(
                    o_out[:, tk, :], fin[:, 0:H], rcp[:]
                )
                if last:
                    # split the final chunk's store so the last piece on
                    # the tail is small
                    nc.gpsimd.dma_start(
                        outR[:, tk:tk + 1, :], o_out[:, tk:tk + 1, :]
                    )
            if not last:
                nc.gpsimd.dma_start(
                    outR[:, ci * 4:(ci + 1) * 4, :],
                    o_out[:, ci * 4:(ci + 1) * 4, :],
                )

        # Interleave: while QKV chunk n waits on its x pieces, the PE
        # queue has attention work for earlier chunks -> no stalls.
        # Output phases are deferred past the next QKV chunk so their
        # DVE chain doesn't head-of-line-block the projection copies.
        emit_qkv(0)
        emit_qkv(1)
        emit_attn_core(0)
        emit_qkv(2)
        emit_attn_out(0)
        emit_attn_core(1)
        emit_qkv(3)
        emit_attn_out(1)
        emit_attn_core(2)
        emit_attn_out(2)
        emit_attn_core(3)
        emit_attn_out(3)


def build_nc(reps=1):
    nc = bacc.Bacc("TRN2", target_bir_lowering=False, debug=False)
    xD = nc.dram_tensor("xT", [C, T], F32, kind="ExternalInput").ap()
    wD = nc.dram_tensor("wqkv", [128, NCT, 192], F32,
                        kind="ExternalInput").ap()
    outD = nc.dram_tensor("out", [T, H], F32, kind="ExternalOutput").ap()

    ALU = mybir.AluOpType
    AF = mybir.ActivationFunctionType

    with tile.TileContext(nc) as tc:
        with (
            tc.tile_pool(name="const", bufs=1) as cpool,
            tc.tile_pool(name="xt", bufs=1) as xtpool,
            tc.tile_pool(name="qk", bufs=2) as qkpool,
            tc.tile_pool(name="pt", bufs=6) as ptpool,
            tc.tile_pool(name="osb", bufs=3) as opool,
            tc.tile_pool(name="fin", bufs=2) as fpool,
        ):
            # zero dummy: PE warm-up source + ACT exp-table priming, ready
            # before any DMA lands.
            dum = cpool.tile([128, 512], BF16)
            nc.gpsimd.memset(dum[:], 0.0)
            prim = cpool.tile([128, 1], F32)

            # packed [Wq|Wk|Wv] per c-tile, host-side layout; one cast DMA
            wqkv = cpool.tile([128, NCT, 192], BF16)
            nc.gpsimd.dma_start(wqkv[:], wD[:])

            # x load descriptors next: everything below is engine-side or
            # rides behind them
            xt0 = emit_loads(nc, xD, xtpool)

            maskd = cpool.tile([128, 128], F32)  # 0 where t>=s else MASKVAL
            nc.gpsimd.memset(maskd[:], 0.0)
            nc.gpsimd.affine_select(
                out=maskd[:], in_=maskd[:],
                compare_op=ALU.is_ge, fill=MASKVAL,
                base=0, pattern=[[1, 128]], channel_multiplier=-1,
            )
            ident = cpool.tile([128, 128], F32)
            make_identity(nc, ident[:])

            # prime the ScalarE activation table before the first real exp
            nc.scalar.activation(prim[:], dum[:, 0:1], AF.Exp, scale=SCALE)

            consts = (wqkv, maskd, ident, dum)
            pools = (qkpool, ptpool, opool, fpool)
            for rep in range(reps):
                xt_cur = xt0 if rep == 0 else emit_loads(nc, xD, xtpool)
                emit_body(nc, tc, outD, consts, pools, xt_cur)

    nc.compile()
    return nc


def _pack_wqkv(Wq, Wk, Wv):
    # [128, NCT, 192]: per c-tile k, cols 0:64 Wq, 64:128 Wk, 128:192 Wv
    w = np.empty((128, NCT, 192), dtype=np.float32)
    for k in range(NCT):
        rows = slice(k * 128, (k + 1) * 128)
        w[:, k, 0:64] = Wq[rows]
        w[:, k, 64:128] = Wk[rows]
        w[:, k, 128:192] = Wv[rows]
    return w


def make_in_maps(x, Wq, Wk, Wv):
    wqkv = _pack_wqkv(
        np.asarray(Wq, dtype=np.float32),
        np.asarray(Wk, dtype=np.float32),
        np.asarray(Wv, dtype=np.float32),
    )
    return [
        {
            "xT": np.ascontiguousarray(
                np.asarray(x[b], dtype=np.float32).T
            ),
            "wqkv": wqkv,
        }
        for b in range(B)
    ]


_NC = None


def kernel(x, Wq, Wk, Wv):
    global _NC
    if _NC is None:
        _NC = build_nc()
    in_maps = make_in_maps(x, Wq, Wk, Wv)
    res = run_bass_kernel_spmd(_NC, in_maps, core_ids=list(range(B)))
    return np.stack([res.results[b]["out"] for b in range(B)], axis=0)
